# revision 12
# baseline (speedup 1.0000x reference)
"""Balanced BCE loss on 8 Trainium2 NeuronCores.

loss = -sum_i [ beta_i * sum_j(t_ij * ln(p_ij))
                + (1-beta_i) * sum_j((1-t_ij) * ln(1-p_ij)) ]
beta_i = 1 - mean_j(t_ij)

Per-core row statistics (8 batch rows per core):
  S=sum(t)  A=sum(t*lnp)  C=sum(t*ln1mp)  B=sum(ln1mp)
host combines: loss = -sum_rows[ beta*A + (1-beta)*(B-C) ], beta = 1-S/N

Engine assignment per virtual row tile [128, w]:
  - ACT: lnp = Ln(p) bf16; ln1mp = Ln(1-p) bf16 (no accum - it is slow)
  - DVE: cast t->bf16 (2x); m1 = t*lnp, m2 = t*ln1mp (bf16 TT, 2x)
  - GpSimd (otherwise idle): whole-tile XYZWC reduces of t and ln1mp
    give the scalars S and B directly (no cross-partition fold needed)
  - PE: one-hot-weight chunk matmuls accumulate A and C into
    per-virtual-row PSUM partitions psX[NV, 512], banks interleaved
    A,C,A,C so consecutive matmuls never hit the same PSUM bank.
  - End: two DVE X-reduces [NV, 512] -> [NV, 1] give A and C.

DMA schedule: 28 input DMAs on the SP HWDGE queue.  A DMA trigger only
admits when the 9-back predecessor completes (9 completion-semaphore
lanes), so sizes are ordered thin-first: row 0 as 4 column-quarters
(their early completions admit the fat middle), rows 1-6 as full 1MB
tiles, row 7 as 4 quarters so the compute tail after the final DMA is
short.  Every tile is resident in SBUF simultaneously (no recycling).
"""

from contextlib import ExitStack

import numpy as np

import concourse.bass as bass
import concourse.mybir as mybir
import concourse.tile as tile
from concourse import bacc
from concourse.bass_utils import run_bass_kernel_spmd

B, N = 64, 262144
NCORES = 8
ROWS = B // NCORES  # rows per core
P = 128  # SBUF partitions
F = N // P  # 2048 elements per partition per row
QF = F // 4  # quarter-row width
NV = 14  # virtual rows: 4 quarters of row0, rows 1..6, 4 quarters of row7
CH = 512  # matmul moving-dim chunk (max for f32 PSUM)

AF = mybir.ActivationFunctionType
ALU = mybir.AluOpType
AX = mybir.AxisListType
f32 = mybir.dt.float32
bf16 = mybir.dt.bfloat16

# test.py can flip this to capture an NTFF profile of the run
TRACE = False
LAST = None  # BassKernelResults of the most recent kernel() call


def _emit(tc, out_ac, out_sb, inp_ap, tgt_ap):
    nc = tc.nc

    with ExitStack() as ctx:
        io_pool = ctx.enter_context(tc.tile_pool(name="io", bufs=6))
        ioq_pool = ctx.enter_context(tc.tile_pool(name="ioq", bufs=8))
        bf_pool = ctx.enter_context(tc.tile_pool(name="bf", bufs=2))
        tb_pool = ctx.enter_context(tc.tile_pool(name="tb", bufs=3))
        psum_pool = ctx.enter_context(tc.tile_pool(name="ps", bufs=1, space="PSUM"))
        singles = ctx.enter_context(tc.tile_pool(name="const", bufs=1))

        # one-hot stationary matrices: block v is [128, NV] bf16 with
        # column v all-ones (routes vrow v's partition sums to psum row v)
        oh = singles.tile([P, NV * NV], bf16, tag="oh")
        nc.vector.memset(oh[:], 0.0)
        for v in range(NV):
            nc.vector.memset(oh[:, v * NV + v : v * NV + v + 1], 1.0)
        ac_sb = singles.tile([NV, 2], f32, tag="ac_sb")
        # S block cols 0:NV, B block NV:2NV (written as scalars by GpSimd)
        sb_sb = singles.tile([1, 2 * NV], f32, tag="sb_sb")

        inp3 = inp_ap.rearrange("r (p f) -> p r f", p=P)
        tgt3 = tgt_ap.rearrange("r (p f) -> p r f", p=P)

        psA = psum_pool.tile([NV, CH], f32, tag="psA", name="psA")
        psC = psum_pool.tile([NV, CH], f32, tag="psC", name="psC")

        # virtual rows in DMA/compute order (col, dram_row, offset, width)
        vrows = [(q, 0, q * QF, QF) for q in range(4)]
        vrows += [(3 + r, r, 0, F) for r in range(1, ROWS - 1)]
        vrows += [(10 + q, ROWS - 1, q * QF, QF) for q in range(4)]

        ptiles, ttiles = [], []
        for col, r, off, w in vrows:
            pool = io_pool if w == F else ioq_pool
            pp = pool.tile([P, w], f32, tag=f"p{w}", name=f"pp_{col}")
            nc.sync.dma_start(pp[:], inp3[:, r, off : off + w])
            ptiles.append(pp)
            tt = pool.tile([P, w], f32, tag=f"t{w}", name=f"tt_{col}")
            nc.sync.dma_start(tt[:], tgt3[:, r, off : off + w])
            ttiles.append(tt)

        nvr = len(vrows)
        for i, (col, r, off, w) in enumerate(vrows):
            p_t = ptiles[i][:]
            t_t = ttiles[i][:]
            nch = w // CH

            logp = bf_pool.tile([P, w], bf16, tag=f"logp{w}")
            nc.scalar.activation(logp[:], p_t, AF.Ln)
            l1mp = bf_pool.tile([P, w], bf16, tag=f"l1mp{w}")
            nc.scalar.activation(l1mp[:], p_t, AF.Ln, scale=-1.0, bias=1.0)

            # S and B as whole-tile scalars on the otherwise idle GpSimd
            nc.gpsimd.tensor_reduce(
                sb_sb[0:1, col : col + 1], t_t, axis=AX.XYZWC, op=ALU.add
            )
            nc.gpsimd.tensor_reduce(
                sb_sb[0:1, NV + col : NV + col + 1], l1mp[:],
                axis=AX.XYZWC, op=ALU.add,
            )

            tb = tb_pool.tile([P, w], bf16, tag=f"tb{w}")
            nc.vector.tensor_copy(tb[:], t_t)
            m1 = bf_pool.tile([P, w], bf16, tag=f"m{w}")
            nc.vector.tensor_mul(m1[:], tb[:], logp[:])
            m2 = bf_pool.tile([P, w], bf16, tag=f"m{w}")
            nc.vector.tensor_mul(m2[:], tb[:], l1mp[:])

            ohv = oh[:, col * NV : (col + 1) * NV]
            for c in range(nch):
                for ps, src in ((psA, m1), (psC, m2)):
                    nc.tensor.matmul(
                        ps[:, :],
                        ohv,
                        src[:, c * CH : (c + 1) * CH],
                        start=(i == 0 and c == 0),
                        stop=(i == nvr - 1 and c == nch - 1),
                        skip_group_check=True,
                    )

        for j, ps in enumerate((psA, psC)):
            nc.vector.tensor_reduce(
                ac_sb[:, j : j + 1], ps[:, :], axis=AX.X, op=ALU.add
            )
        nc.sync.dma_start(out_ac, ac_sb[:])
        nc.sync.dma_start(out_sb, sb_sb[:])


_PROG_CACHE = {}


def _build_program():
    key = (ROWS, N)
    if key not in _PROG_CACHE:
        nc = bacc.Bacc("TRN2", target_bir_lowering=False, debug=False)
        inp = nc.dram_tensor("input", [ROWS, N], f32, kind="ExternalInput").ap()
        tgt = nc.dram_tensor("target", [ROWS, N], f32, kind="ExternalInput").ap()
        oac = nc.dram_tensor("partialsAC", [NV, 2], f32, kind="ExternalOutput").ap()
        osb = nc.dram_tensor("partialsSB", [1, 2 * NV], f32, kind="ExternalOutput").ap()
        with tile.TileContext(nc) as tc:
            _emit(tc, oac, osb, inp, tgt)
        nc.finalize()
        _PROG_CACHE[key] = nc
    return _PROG_CACHE[key]


def kernel(input, target):
    global LAST
    input = np.ascontiguousarray(np.asarray(input))
    target = np.ascontiguousarray(np.asarray(target))
    assert input.shape == (B, N) and target.shape == (B, N)

    nc = _build_program()
    in_maps = [
        {
            "input": input[c * ROWS : (c + 1) * ROWS],
            "target": target[c * ROWS : (c + 1) * ROWS],
        }
        for c in range(NCORES)
    ]
    res = run_bass_kernel_spmd(nc, in_maps, core_ids=list(range(NCORES)), trace=TRACE)
    LAST = res

    # vrow -> batch row mapping: cols 0-3 = row0 quarters, 4-9 = rows 1-6,
    # 10-13 = row7 quarters
    def fold(v):  # v: [NV] per-vrow partials -> [8] per-row
        return np.concatenate([[v[0:4].sum()], v[4:10], [v[10:14].sum()]])

    total = np.float64(0.0)
    for c in range(NCORES):
        ac = res.results[c]["partialsAC"].astype(np.float64)  # [NV, 2]
        sb = res.results[c]["partialsSB"].astype(np.float64).reshape(2, NV)
        S, Bv = fold(sb[0]), fold(sb[1])
        A, C = fold(ac[:, 0]), fold(ac[:, 1])
        beta = 1.0 - S / N
        total += np.sum(beta * A + (1.0 - beta) * (Bv - C))
    return np.float32(-total)


# revision 13
# speedup vs baseline: 2.1640x; 2.1640x over previous
"""Balanced BCE loss on 8 Trainium2 NeuronCores.

loss = -sum_i [ beta_i * sum_j(t_ij * ln(p_ij))
                + (1-beta_i) * sum_j((1-t_ij) * ln(1-p_ij)) ]
beta_i = 1 - mean_j(t_ij)

Per-core row statistics (8 batch rows per core):
  S=sum(t)  A=sum(t*lnp)  C=sum(t*ln1mp)  B=sum(ln1mp)
host combines: loss = -sum_rows[ beta*A + (1-beta)*(B-C) ], beta = 1-S/N

Engine assignment per row tile [128, F]:
  - ACT: lnp = Ln(p) bf16; ln1mp = Ln(1-p) bf16 with accum_out -> B
    (the accumulator read is the only extra ACT cost; ACT's two Ln
    passes are already at the DMA-stream pace, so nothing else here)
  - DVE: cast t->bf16 (2x); m1 = t*lnp, m2 = t*ln1mp (bf16 TT, 2x)
  - PE: one-hot-weight chunk matmuls accumulate S, A and C into
    per-row PSUM partitions psX[8, 512]; banks rotate S,A,C per chunk
    so consecutive matmuls never hit the same PSUM bank.
  - End: A/C X-reduced on DVE, S on ACT (copy+accum), B folded with
    one tiny f32 matmul.

DMA schedule: row 0 is fetched as 8 quarter-size sub-DMAs (into two
full-row tiles) so the 9-lane completion window fills with small
transfers whose early completions admit the 14 full-row DMAs behind
them; every tile is resident in SBUF (no recycling stalls).  p7 is
fetched before t7 so the final ACT Ln pair starts one transfer early.
"""

from contextlib import ExitStack

import numpy as np

import concourse.bass as bass
import concourse.mybir as mybir
import concourse.tile as tile
from concourse import bacc
from concourse.bass_utils import run_bass_kernel_spmd

B, N = 64, 262144
NCORES = 8
ROWS = B // NCORES  # rows per core
P = 128  # SBUF partitions
F = N // P  # 2048 elements per partition per row
QF = F // 4
CH = 512  # matmul moving-dim chunk (max for f32 PSUM)

AF = mybir.ActivationFunctionType
ALU = mybir.AluOpType
AX = mybir.AxisListType
f32 = mybir.dt.float32
bf16 = mybir.dt.bfloat16

# test.py can flip this to capture an NTFF profile of the run
TRACE = False
LAST = None  # BassKernelResults of the most recent kernel() call


def _emit(tc, out_ac, out_sb, inp_ap, tgt_ap):
    nc = tc.nc
    rows = ROWS

    with ExitStack() as ctx:
        io_pool = ctx.enter_context(tc.tile_pool(name="io", bufs=rows))
        bf_pool = ctx.enter_context(tc.tile_pool(name="bf", bufs=2))
        tb_pool = ctx.enter_context(tc.tile_pool(name="tb", bufs=3))
        psum_pool = ctx.enter_context(tc.tile_pool(name="ps", bufs=1, space="PSUM"))
        singles = ctx.enter_context(tc.tile_pool(name="const", bufs=1))

        # one-hot stationary matrices: block r is [128, 8] bf16 with
        # column r all-ones (routes row r's partition sums to psum row r)
        oh = singles.tile([P, rows * rows], bf16, tag="oh")
        nc.vector.memset(oh[:], 0.0)
        for v in range(rows):
            nc.vector.memset(oh[:, v * rows + v : v * rows + v + 1], 1.0)
        ones_f = singles.tile([P, 1], f32, tag="ones_f")
        nc.vector.memset(ones_f[:], 1.0)
        accB = singles.tile([P, rows], f32, tag="accB")
        ac_sb = singles.tile([rows, 3], f32, tag="ac_sb")  # cols: S, A, C
        sb_sb = singles.tile([1, rows], f32, tag="sb_sb")  # B row
        junk = singles.tile([rows, CH], f32, tag="junk")

        inp3 = inp_ap.rearrange("r (p f) -> p r f", p=P)
        tgt3 = tgt_ap.rearrange("r (p f) -> p r f", p=P)

        psS = psum_pool.tile([rows, CH], f32, tag="psS", name="psS")
        psA = psum_pool.tile([rows, CH], f32, tag="psA", name="psA")
        psC = psum_pool.tile([rows, CH], f32, tag="psC", name="psC")
        psB = psum_pool.tile([1, rows], f32, tag="psB", name="psB")

        # input DMAs: row 0 in quarter strips (fast early completions fill
        # the HWDGE admission window), rows 1-7 full; p before t each row
        ptiles, ttiles = [], []
        for r in range(rows):
            pp = io_pool.tile([P, F], f32, tag="p", name=f"pp_{r}")
            tt = io_pool.tile([P, F], f32, tag="t", name=f"tt_{r}")
            if r == 0:
                for q in range(4):
                    sl = slice(q * QF, (q + 1) * QF)
                    nc.sync.dma_start(pp[:, sl], inp3[:, r, sl])
                    nc.sync.dma_start(tt[:, sl], tgt3[:, r, sl])
            else:
                nc.sync.dma_start(pp[:], inp3[:, r, :])
                nc.sync.dma_start(tt[:], tgt3[:, r, :])
            ptiles.append(pp)
            ttiles.append(tt)

        nch = F // CH
        for r in range(rows):
            p_t = ptiles[r][:]
            t_t = ttiles[r][:]

            logp = bf_pool.tile([P, F], bf16, tag="logp")
            nc.scalar.activation(logp[:], p_t, AF.Ln)
            l1mp = bf_pool.tile([P, F], bf16, tag="l1mp")
            nc.scalar.activation(
                l1mp[:], p_t, AF.Ln, scale=-1.0, bias=1.0,
                accum_out=accB[:, r : r + 1],
            )

            tb = tb_pool.tile([P, F], bf16, tag="tb")
            nc.vector.tensor_copy(tb[:], t_t)
            m1 = bf_pool.tile([P, F], bf16, tag="m1")
            nc.vector.tensor_mul(m1[:], tb[:], logp[:])
            m2 = bf_pool.tile([P, F], bf16, tag="m2")
            nc.vector.tensor_mul(m2[:], tb[:], l1mp[:])

            ohv = oh[:, r * rows : (r + 1) * rows]
            for c in range(nch):
                for ps, src in ((psS, tb), (psA, m1), (psC, m2)):
                    nc.tensor.matmul(
                        ps[:, :],
                        ohv,
                        src[:, c * CH : (c + 1) * CH],
                        start=(r == 0 and c == 0),
                        stop=(r == rows - 1 and c == nch - 1),
                        skip_group_check=True,
                    )

        # second level: S on ACT (copy+accum), A/C on DVE, B-fold on PE
        nc.scalar.activation(
            junk[:, :], psS[:, :], AF.Copy, accum_out=ac_sb[:, 0:1]
        )
        nc.vector.tensor_reduce(ac_sb[:, 1:2], psA[:, :], axis=AX.X, op=ALU.add)
        nc.vector.tensor_reduce(ac_sb[:, 2:3], psC[:, :], axis=AX.X, op=ALU.add)
        nc.tensor.matmul(psB[0:1, :], ones_f[:], accB[:, :], start=True, stop=True)
        nc.vector.tensor_copy(sb_sb[0:1, :], psB[0:1, :])
        nc.sync.dma_start(out_ac, ac_sb[:])
        nc.sync.dma_start(out_sb, sb_sb[:])


_PROG_CACHE = {}


def _build_program():
    key = (ROWS, N)
    if key not in _PROG_CACHE:
        nc = bacc.Bacc("TRN2", target_bir_lowering=False, debug=False)
        inp = nc.dram_tensor("input", [ROWS, N], f32, kind="ExternalInput").ap()
        tgt = nc.dram_tensor("target", [ROWS, N], f32, kind="ExternalInput").ap()
        oac = nc.dram_tensor("partialsAC", [ROWS, 3], f32, kind="ExternalOutput").ap()
        osb = nc.dram_tensor("partialsSB", [1, ROWS], f32, kind="ExternalOutput").ap()
        with tile.TileContext(nc) as tc:
            _emit(tc, oac, osb, inp, tgt)
        nc.finalize()
        _PROG_CACHE[key] = nc
    return _PROG_CACHE[key]


def kernel(input, target):
    global LAST
    input = np.ascontiguousarray(np.asarray(input))
    target = np.ascontiguousarray(np.asarray(target))
    assert input.shape == (B, N) and target.shape == (B, N)

    nc = _build_program()
    in_maps = [
        {
            "input": input[c * ROWS : (c + 1) * ROWS],
            "target": target[c * ROWS : (c + 1) * ROWS],
        }
        for c in range(NCORES)
    ]
    res = run_bass_kernel_spmd(nc, in_maps, core_ids=list(range(NCORES)), trace=TRACE)
    LAST = res

    total = np.float64(0.0)
    for c in range(NCORES):
        ac = res.results[c]["partialsAC"].astype(np.float64)  # [8, 3] = S, A, C
        Bv = res.results[c]["partialsSB"].astype(np.float64).reshape(ROWS)
        S, A, C = ac[:, 0], ac[:, 1], ac[:, 2]
        beta = 1.0 - S / N
        total += np.sum(beta * A + (1.0 - beta) * (Bv - C))
    return np.float32(-total)
